# revision 10
# baseline (speedup 1.0000x reference)
"""Trainium2 Bass kernel v2 for multi-head attention (nn_Attention_61168924230279).

Differences vs v1 baseline:
  - Q/K projected directly in transposed [e, m] layout (stationary = weight
    chunk, stream = xT columns): no PE transposes, no QT/KT copies.
  - Bias + NeoX rope fused into 3 DVE scalar_tensor_tensor ops reading the
    projection PSUM directly (cos/sign-folded-sin tables pre-transposed).
  - bf16 on the attention/value/output path (QT/KT/V/at/outc/wo/y-partials);
    x and wq/wk/wv stay f32r so logits keep full input precision.
  - Softmax denominators via an exact f32 Asum of the bf16 at tiles on DVE
    (bf16+bf16->f32 adds) + one ones-matmul per (qc, h) on PE, replacing the
    per-key-tile PE l matmuls (-51us of PE time). gpsimd is avoided for bulk
    math (measured far slower on hw than the cost model predicts).
  - Software pipelining at batch granularity: the previous batch's attention
    (phase B) + output projection (phase C) are emitted as a generator and
    pumped between projection units of the current batch, keeping ACT's exp
    stream overlapped with PE's projection stream; exp production runs two
    key-tiles ahead of its consumers; each qc's output-projection tiles are
    deferred into the next qc's loop so ACT evictions stay evenly spread.
  - y partials evicted PSUM->bf16 on ACT (split ACT/DVE in the final drain)
    and DMA'd as bf16; host sums the 8 bf16 partials in f64.

Sharding: tensor-parallel over heads. 8 cores x 2 heads each; partials are
summed on the host (+bo, +bv@wo.T since attention rows sum to 1).
"""

import math
import os
from contextlib import ExitStack

import numpy as np

import concourse.bass as bass
import concourse.tile as tile
from concourse import bacc, mybir

P = 128
D = 2048
ND = D // P            # 16 contraction chunks
B = 2
S = 2048
M = B * S              # 4096
NMT = S // P           # 16 m-tiles per batch
MC = 256               # m-chunk width for transposed projections
NMC = S // MC          # 8 m-chunks per batch
HD = 128
H2 = HD // 2
HPC = 2                # heads per core
E2 = HPC * HD          # 256 (v width per core)
E4 = 2 * E2            # 512 (q|k width per core)
NQC = S // 512         # 4 query chunks per batch
NKT = S // P           # 16 key tiles per batch
N_CORES = 8
SCALE = 1.0 / math.sqrt(HD)
ROPE_THETA = 10000.0

F32 = mybir.dt.float32
F32R = mybir.dt.float32r
BF16 = mybir.dt.bfloat16

ADD = mybir.AluOpType.add
MULT = mybir.AluOpType.mult


def _pump(gen, n=1):
    if gen is None:
        return
    for _ in range(n):
        try:
            next(gen)
        except StopIteration:
            return


def _emit(nc, tc, t):
    with ExitStack() as ctx:
        ec_ = ctx.enter_context
        const = ec_(tc.tile_pool(name="const", bufs=1))
        wpool = ec_(tc.tile_pool(name="weights", bufs=1))
        tabs = ec_(tc.tile_pool(name="tables", bufs=1))
        xtp = ec_(tc.tile_pool(name="xt", bufs=int(os.environ.get("K_XTB", "3"))))
        qtkv = ec_(tc.tile_pool(name="qtkv", bufs=2))
        up = ec_(tc.tile_pool(name="u", bufs=2))
        attnp = ec_(tc.tile_pool(name="attn", bufs=int(os.environ.get("K_ATB", "11"))))
        asump = ec_(tc.tile_pool(name="asum", bufs=int(os.environ.get("K_ASB", "4"))))
        rbp = ec_(tc.tile_pool(name="rb", bufs=2))
        rsp = ec_(tc.tile_pool(name="rs", bufs=2))
        outp = ec_(tc.tile_pool(name="outT", bufs=2))
        avsbp = ec_(tc.tile_pool(name="avsb", bufs=2))
        yp = ec_(tc.tile_pool(name="y", bufs=int(os.environ.get("K_YB", "4"))))
        psA = ec_(tc.tile_pool(name="psA", bufs=int(os.environ.get("K_PSA", "6")), space="PSUM"))
        psAv = ec_(tc.tile_pool(name="psAv", bufs=int(os.environ.get("K_PSAV", "2")), space="PSUM"))

        # --- constants / weights / tables (loaded once) ---
        ones_c32 = const.tile([P, 1], F32)
        nc.vector.memset(ones_c32, 1.0)
        ones_c = const.tile([P, 1], F32R)
        nc.vector.tensor_copy(ones_c, ones_c32)
        ones_r32 = const.tile([1, P], F32)
        nc.vector.memset(ones_r32, 1.0)
        ones_r = const.tile([1, P], F32R)
        nc.vector.tensor_copy(ones_r, ones_r32)
        bqk_sb = const.tile([P, 4], F32)
        nc.sync.dma_start(bqk_sb, t["bqk4"])
        # half-swapped bias copy: the rope u-ops read the projection PSUM at
        # the opposite partition half, and the BIR verifier requires all SBUF
        # operands of TensorScalarPtr to share a base partition.
        bqk_sw = const.tile([P, 4], F32)
        nc.sync.dma_start(bqk_sw, t["bqk4s"])

# Weight loads split per d-chunk so the first projection chain can
        # start as soon as its chunk lands (instead of waiting ~27us for
        # monolithic 4MB+ transfers). Ordered by first use.
        wqk_s = wpool.tile([P, ND, E4], F32R)
        wqk_r = t["wqkT"].rearrange("(k p) e -> p k e", p=P)
        for k in range(ND):
            nc.gpsimd.dma_start(wqk_s[:, k, :], wqk_r[:, k, :])
        wv_s = wpool.tile([P, ND, E2], F32R)
        wv_r = t["wvT"].rearrange("(k p) e -> p k e", p=P)
        for k in range(ND):
            nc.gpsimd.dma_start(wv_s[:, k, :], wv_r[:, k, :])
        wo_s = wpool.tile([P, HPC, D], BF16)
        nc.gpsimd.dma_start(wo_s, t["woT"].rearrange("(h p) d -> p h d", p=P))
        # rope tables go on the sync queue AFTER the first xt chunk (see
        # emit_A), so the first projection's input isn't queued behind 2MB of
        # tables; first rope needs them only ~2 proj units later.
        cosT = tabs.tile([P, S], F32)
        sinT = tabs.tile([P, S], F32)
        startup = [lambda: nc.scalar.dma_start(cosT, t["cosT"]),
                   lambda: nc.scalar.dma_start(sinT, t["sinT"])]

        def b_units(b, QT, KT, V, is_last=False):
            """Phase B (attention) + phase C (output proj) for one batch.

            Within a query chunk, exp(logits) production runs one key-tile
            ahead of its consumers (av matmul + Asum accumulate), so neither
            PE nor DVE ever waits on the ACT exp latency. The softmax
            denominator comes from an exact f32 Asum of the bf16 at tiles on
            DVE (bf16+bf16->f32 adds), finished by a single ones-matmul per
            (qc, h) on PE -- this removes the per-key-tile l matmuls that
            cost ~55us of PE time in the baseline.
            """
            deferred_y = []
            for qc in range(NQC):
                qsl = slice(qc * 512, (qc + 1) * 512)
                av_ps, Asum, l_ps = {}, {}, {}
                for h in range(HPC):
                    av_ps[h] = psAv.tile([P, 512], F32, tag="av", name=f"av_{b}_{qc}_{h}")
                    Asum[h] = asump.tile([P, 512], F32R, tag="as", name=f"as_{b}_{qc}_{h}")
                ats_by_kt = {}

                def lg_exp(kt):
                    ats = {}
                    for h in range(HPC):
                        lg = psA.tile([P, 512], F32, tag="ps", name=f"lg_{b}_{qc}_{kt}_{h}")
                        nc.tensor.matmul(
                            lg, KT[:, h, kt * P:(kt + 1) * P], QT[:, h, qsl],
                            start=True, stop=True,
                        )
                        at = attnp.tile([P, 512], BF16, tag="at", name=f"at_{b}_{qc}_{kt}_{h}")
                        nc.scalar.activation(
                            at, lg, mybir.ActivationFunctionType.Exp, scale=SCALE
                        )
                        ats[h] = at
                    ats_by_kt[kt] = ats

                def consume(kt):
                    for h in range(HPC):
                        nc.tensor.matmul(
                            av_ps[h], V[:, kt, h * HD:(h + 1) * HD], ats_by_kt[kt][h],
                            start=(kt == 0), stop=(kt == NKT - 1),
                        )
                    for h in range(HPC):
                        # h0 chain on DVE, h1 on gpsimd (K_ASENG=split), or
                        # both on DVE: gpsimd's real f32 throughput is the
                        # least-trusted part of the cost model.
                        mode = os.environ.get("K_ASENG", "dve")
                        eng = nc.vector if (mode == "dve" or h == 0) else nc.gpsimd
                        if kt == 1:
                            eng.tensor_add(Asum[h], ats_by_kt[0][h], ats_by_kt[1][h])
                        elif kt > 1:
                            eng.tensor_add(Asum[h], Asum[h], ats_by_kt[kt][h])
                    if kt >= 2:
                        ats_by_kt.pop(kt - 2, None)

                LAG = int(os.environ.get("K_LAG", "2"))
                for kt in range(NKT):
                    lg_exp(kt)
                    if kt >= LAG:
                        consume(kt - LAG)
                    # previous qc's deferred y tiles, one per unit, so their
                    # ACT evictions interleave with this qc's exp stream
                    # instead of bunching up at the final.
                    if deferred_y:
                        deferred_y.pop(0)()
                    yield
                for kk in range(NKT - LAG, NKT):
                    consume(kk)
                    yield
                outc = outp.tile([P, HPC, 512], BF16, tag="outc", name=f"outc_{b}_{qc}")
                # av psum -> SBUF bf16 on ACT first: it only depends on the
                # last av matmul, so it overlaps the l/recip/rb chain, frees
                # the av bank early, and leaves rb as the only PSUM input of
                # the outc multiply.
                av_sb = {}
                for h in range(HPC):
                    av_sb[h] = avsbp.tile([P, 512], BF16, tag="avsb", name=f"avsb_{b}_{qc}_{h}")
                    nc.scalar.copy(av_sb[h], av_ps[h])
                # parallelize the two h-chains across engines: both l matmuls
                # back-to-back on PE, then both recips (DVE), rb broadcasts
                # (PE rank-1 matmul), outc muls (DVE).
                for h in range(HPC):
                    l_ps[h] = psA.tile([1, 512], F32, tag="ps", name=f"l_{b}_{qc}_{h}")
                    nc.tensor.matmul(l_ps[h], ones_c, Asum[h], start=True, stop=True)
                rss = {}
                use_pe_rb = os.environ.get("K_RB", "gp") == "pe"
                for h in range(HPC):
                    if use_pe_rb:
                        rss[h] = rsp.tile([1, 512], F32R, tag="rsr", name=f"rs_{b}_{qc}_{h}")
                        with nc.allow_low_precision(reason="f32r rounding of 1/l"):
                            nc.vector.reciprocal(rss[h], l_ps[h])
                    else:
                        rss[h] = rsp.tile([1, 512], F32, tag="rs", name=f"rs_{b}_{qc}_{h}")
                        nc.vector.reciprocal(rss[h], l_ps[h])
                rbs = {}
                for h in range(HPC):
                    if use_pe_rb:
                        # broadcast 1/l across partitions via a rank-1 PE
                        # matmul (gpsimd partition_broadcast is much slower on
                        # real hw than the cost model suggests, and it sits on
                        # the critical path to the y matmuls).
                        rbs[h] = psA.tile([P, 512], F32, tag="ps", name=f"rb_{b}_{qc}_{h}")
                        nc.tensor.matmul(rbs[h], ones_r, rss[h], start=True, stop=True)
                    else:
                        rbs[h] = rbp.tile([P, 512], F32, tag="rb", name=f"rb_{b}_{qc}_{h}")
                        nc.gpsimd.partition_broadcast(rbs[h], rss[h])
                for h in range(HPC):
                    nc.vector.tensor_mul(outc[:, h, :], av_sb[h], rbs[h])
                # pipeline distance: let a few projection units run on PE while
                # the recip->broadcast->outc chain completes, so the y matmuls
                # below don't stall the PE queue.
                for _ in range(int(os.environ.get("K_FYLD", "2"))):
                    yield

                def make_y(qc, outc, yu):
                    def emit_y():
                        mtl, och = divmod(yu, 2)
                        mt = qc * 4 + mtl
                        lsl = slice(mtl * P, (mtl + 1) * P)
                        for oc in (2 * och, 2 * och + 1):
                            osl = slice(oc * 512, (oc + 1) * 512)
                            y_ps = psA.tile([P, 512], F32, tag="ps", name=f"yp_{b}_{mt}_{oc}")
                            nc.tensor.matmul(
                                y_ps, outc[:, 0, lsl], wo_s[:, 0, osl],
                                start=True, stop=False,
                            )
                            nc.tensor.matmul(
                                y_ps, outc[:, 1, lsl], wo_s[:, 1, osl],
                                start=False, stop=True,
                            )
                            yt = yp.tile([P, 512], BF16, tag="yt", name=f"yt_{b}_{mt}_{oc}")
                            # in the final drain there is no projection work to
                            # hide behind, so split evictions ACT/DVE there.
                            if is_last and oc % 2 == 1:
                                nc.vector.tensor_copy(yt, y_ps)
                            else:
                                nc.scalar.copy(yt, y_ps)
                            nc.sync.dma_start(
                                t["y"][b * S + mt * P: b * S + (mt + 1) * P, osl], yt
                            )
                    return emit_y

                deferred_y = [make_y(qc, outc, yu) for yu in range(8)]
            # tail: the last qc's y tiles drain at the end of this batch's
            # generator, overlapping the next batch's projection stream.
            for _ in range(int(os.environ.get("K_TYLD", "3"))):
                yield
            for fn_ in deferred_y:
                fn_()
                yield

        def emit_A(b, pending):
            """Projections + rope for batch b, pumping the previous batch's
            attention units between projection units."""
            QT = qtkv.tile([P, HPC, S], BF16, tag="QT", name=f"QT_{b}")
            KT = qtkv.tile([P, HPC, S], BF16, tag="KT", name=f"KT_{b}")
            V = qtkv.tile([P, NMT, E2], BF16, tag="V", name=f"V_{b}")
            for mc in range(NMC):
                gm = b * S + mc * MC
                xt = xtp.tile([P, ND, MC], F32R, tag="xt", name=f"xt_{b}_{mc}")
                nc.sync.dma_start(
                    xt, t["xT"][:, gm:gm + MC].rearrange("(k p) m -> p k m", p=P).bitcast(F32R)
                )
                while startup:
                    startup.pop(0)()
                msl = slice(mc * MC, (mc + 1) * MC)
                for ecn in range(4):
                    ps = psA.tile([P, MC], F32, tag="ps", name=f"qk_{b}_{mc}_{ecn}")
                    for k in range(ND):
                        nc.tensor.matmul(
                            ps, wqk_s[:, k, ecn * P:(ecn + 1) * P], xt[:, k, :],
                            start=(k == 0), stop=(k == ND - 1),
                        )
                    _pump(pending)
                    dst = (QT if ecn < 2 else KT)[:, ecn % 2, msl]
                    bias = bqk_sb[:, ecn:ecn + 1]
                    # dst = (ps + bias) * cos ; u = swap_halves(ps + bias) * sin'
                    nc.vector.scalar_tensor_tensor(
                        dst, ps, bias, cosT[:, msl], ADD, MULT
                    )
                    u = up.tile([P, MC], BF16, tag="u", name=f"u_{b}_{mc}_{ecn}")
                    nc.vector.scalar_tensor_tensor(
                        u[0:H2, :], ps[H2:P, :], bqk_sw[0:H2, ecn:ecn + 1],
                        sinT[0:H2, msl], ADD, MULT,
                    )
                    nc.vector.scalar_tensor_tensor(
                        u[H2:P, :], ps[0:H2, :], bqk_sw[H2:P, ecn:ecn + 1],
                        sinT[H2:P, msl], ADD, MULT,
                    )
                    nc.vector.tensor_add(dst, dst, u)
                    _pump(pending)
                for mtl in range(MC // P):
                    mt = mc * (MC // P) + mtl
                    vps = psA.tile([P, E2], F32, tag="ps", name=f"v_{b}_{mt}")
                    for k in range(ND):
                        nc.tensor.matmul(
                            vps, xt[:, k, mtl * P:(mtl + 1) * P], wv_s[:, k, :],
                            start=(k == 0), stop=(k == ND - 1),
                        )
                    nc.scalar.copy(V[:, mt, :], vps)
                    _pump(pending)
            return QT, KT, V

        reps = int(os.environ.get("K_REPS", "1"))
        pending = None
        for rep in range(reps):
            for b in range(B):
                QT, KT, V = emit_A(b, pending)
                if pending is not None:
                    for _ in pending:  # drain any remainder
                        pass
                pending = b_units(b, QT, KT, V,
                                  is_last=(rep == reps - 1 and b == B - 1))
        for _ in pending:
            pass


def build_program():
    nc = bacc.Bacc(
        "TRN2",
        target_bir_lowering=False,
        debug=False,
        enable_asserts=False,
        num_devices=N_CORES,
    )
    t = {
        "xT": nc.dram_tensor("xT", [D, M], F32, kind="ExternalInput").ap(),
        "wqkT": nc.dram_tensor("wqkT", [D, E4], F32, kind="ExternalInput").ap(),
        "wvT": nc.dram_tensor("wvT", [D, E2], F32, kind="ExternalInput").ap(),
        "woT": nc.dram_tensor("woT", [E2, D], BF16, kind="ExternalInput").ap(),
        "bqk4": nc.dram_tensor("bqk4", [P, 4], F32, kind="ExternalInput").ap(),
        "bqk4s": nc.dram_tensor("bqk4s", [P, 4], F32, kind="ExternalInput").ap(),
        "cosT": nc.dram_tensor("cosT", [P, S], F32, kind="ExternalInput").ap(),
        "sinT": nc.dram_tensor("sinT", [P, S], F32, kind="ExternalInput").ap(),
        "y": nc.dram_tensor("y", [M, D], BF16, kind="ExternalOutput").ap(),
    }
    with tile.TileContext(nc) as tc:
        _emit(nc, tc, t)
    nc.compile()
    return nc


def rope_tables_T():
    inv_freq = 1.0 / (ROPE_THETA ** (np.arange(0, HD, 2, dtype=np.float32) / HD))
    angles = np.outer(np.arange(S, dtype=np.float32), inv_freq)  # [S, 64]
    ang = np.concatenate([angles, angles], axis=-1)  # [S, 128]
    cos = np.cos(ang).astype(np.float32)
    sin = np.sin(ang).astype(np.float32)
    sins = np.concatenate([-sin[:, :H2], sin[:, H2:]], axis=-1)
    return np.ascontiguousarray(cos.T), np.ascontiguousarray(sins.T)


def make_in_maps(x, wq, bq, wk, bk, wv, bv, wo, bo):
    import ml_dtypes
    xT = np.ascontiguousarray(x.reshape(M, D).T)
    cosT, sinT = rope_tables_T()
    maps = []
    for c in range(N_CORES):
        sl = slice(c * E2, (c + 1) * E2)
        bqk = np.concatenate([bq[sl], bk[sl]]).astype(np.float32)
        maps.append({
            "xT": xT,
            "wqkT": np.ascontiguousarray(np.concatenate([wq[sl], wk[sl]], axis=0).T),
            "wvT": np.ascontiguousarray(wv[sl].T),
            "woT": np.ascontiguousarray(wo[:, sl].T).astype(ml_dtypes.bfloat16),
            "bqk4": np.ascontiguousarray(bqk.reshape(4, P).T),
            "bqk4s": np.ascontiguousarray(np.roll(bqk.reshape(4, P), H2, axis=1).T),
            "cosT": cosT,
            "sinT": sinT,
        })
    return maps


_NC = None


def kernel(**inputs) -> np.ndarray:
    global _NC
    inputs = {k: np.ascontiguousarray(np.asarray(v, dtype=np.float32))
              for k, v in inputs.items()}
    if _NC is None:
        _NC = build_program()
    from concourse.bass_utils import run_bass_kernel_spmd

    maps = make_in_maps(**inputs)
    res = run_bass_kernel_spmd(_NC, maps, list(range(N_CORES)))
    y = np.zeros((M, D), np.float64)
    for c in range(N_CORES):
        y += np.asarray(res.results[c]["y"], dtype=np.float64)
    y += inputs["bo"][None, :] + (inputs["bv"].astype(np.float64) @ inputs["wo"].T.astype(np.float64))[None, :]
    return y.astype(np.float32).reshape(B, S, D)


# revision 11
# speedup vs baseline: 4.5496x; 4.5496x over previous
"""Trainium2 Bass kernel v2 for multi-head attention (nn_Attention_61168924230279).

Differences vs v1 baseline:
  - Q/K projected directly in transposed [e, m] layout (stationary = weight
    chunk, stream = xT columns): no PE transposes, no QT/KT copies.
  - Bias + NeoX rope fused into 3 DVE scalar_tensor_tensor ops reading the
    projection PSUM directly (cos/sign-folded-sin tables pre-transposed).
  - bf16 on the attention/value/output path (QT/KT/V/at/outc/wo/y-partials);
    x and wq/wk/wv stay f32r so logits keep full input precision.
  - Softmax denominators via an exact f32 Asum of the bf16 at tiles on DVE
    (bf16+bf16->f32 adds) + one ones-matmul per (qc, h) on PE, replacing the
    per-key-tile PE l matmuls (-51us of PE time). gpsimd is avoided for bulk
    math (measured far slower on hw than the cost model predicts).
  - Software pipelining at batch granularity: the previous batch's attention
    (phase B) + output projection (phase C) are emitted as a generator and
    pumped between projection units of the current batch, keeping ACT's exp
    stream overlapped with PE's projection stream; exp production runs two
    key-tiles ahead of its consumers; each qc's output-projection tiles are
    deferred into the next qc's loop so ACT evictions stay evenly spread.
  - y partials evicted PSUM->bf16 on ACT (split ACT/DVE in the final drain)
    and DMA'd as bf16; host sums the 8 bf16 partials in f64.

Sharding: tensor-parallel over heads. 8 cores x 2 heads each; partials are
summed on the host (+bo, +bv@wo.T since attention rows sum to 1).
"""

import math
import os
from contextlib import ExitStack

import numpy as np

import concourse.bass as bass
import concourse.tile as tile
from concourse import bacc, mybir

P = 128
D = 2048
ND = D // P            # 16 contraction chunks
B = 2
S = 2048
M = B * S              # 4096
NMT = S // P           # 16 m-tiles per batch
MC = 256               # m-chunk width for transposed projections
NMC = S // MC          # 8 m-chunks per batch
HD = 128
H2 = HD // 2
HPC = 2                # heads per core
E2 = HPC * HD          # 256 (v width per core)
E4 = 2 * E2            # 512 (q|k width per core)
NQC = S // 512         # 4 query chunks per batch
NKT = S // P           # 16 key tiles per batch
N_CORES = 8
SCALE = 1.0 / math.sqrt(HD)
ROPE_THETA = 10000.0

F32 = mybir.dt.float32
F32R = mybir.dt.float32r
BF16 = mybir.dt.bfloat16

ADD = mybir.AluOpType.add
MULT = mybir.AluOpType.mult


class _BStream:
    """Pull-driven view of a b_units generator. The generator yields, before
    each unit, the highest m-chunk index of its own batch that must already be
    emitted; pump(avail) emits the next unit iff its dependency is met."""

    def __init__(self, gen):
        self.gen = gen
        self.done = False
        self.dep = -1
        self._advance()

    def _advance(self):
        try:
            self.dep = next(self.gen)
        except StopIteration:
            self.done = True

    def pump(self, avail):
        if self.done or self.dep > avail:
            return False
        self._advance()
        return True


def _emit(nc, tc, t):
    with ExitStack() as ctx:
        ec_ = ctx.enter_context
        const = ec_(tc.tile_pool(name="const", bufs=1))
        wpool = ec_(tc.tile_pool(name="weights", bufs=1))
        tabs = ec_(tc.tile_pool(name="tables", bufs=1))
        xtp = ec_(tc.tile_pool(name="xt", bufs=int(os.environ.get("K_XTB", "3"))))
        qtkv = ec_(tc.tile_pool(name="qtkv", bufs=2))
        up = ec_(tc.tile_pool(name="u", bufs=2))
        attnp = ec_(tc.tile_pool(name="attn", bufs=int(os.environ.get("K_ATB", "11"))))
        asump = ec_(tc.tile_pool(name="asum", bufs=int(os.environ.get("K_ASB", "4"))))
        rbp = ec_(tc.tile_pool(name="rb", bufs=2))
        rsp = ec_(tc.tile_pool(name="rs", bufs=2))
        outp = ec_(tc.tile_pool(name="outT", bufs=2))
        avsbp = ec_(tc.tile_pool(name="avsb", bufs=2))
        yp = ec_(tc.tile_pool(name="y", bufs=int(os.environ.get("K_YB", "4"))))
        psA = ec_(tc.tile_pool(name="psA", bufs=int(os.environ.get("K_PSA", "6")), space="PSUM"))
        psAv = ec_(tc.tile_pool(name="psAv", bufs=int(os.environ.get("K_PSAV", "2")), space="PSUM"))

        # --- constants / weights / tables (loaded once) ---
        ones_c32 = const.tile([P, 1], F32)
        nc.vector.memset(ones_c32, 1.0)
        ones_c = const.tile([P, 1], F32R)
        nc.vector.tensor_copy(ones_c, ones_c32)
        ones_r32 = const.tile([1, P], F32)
        nc.vector.memset(ones_r32, 1.0)
        ones_r = const.tile([1, P], F32R)
        nc.vector.tensor_copy(ones_r, ones_r32)
        bqk_sb = const.tile([P, 4], F32)
        nc.sync.dma_start(bqk_sb, t["bqk4"])
        # half-swapped bias copy: the rope u-ops read the projection PSUM at
        # the opposite partition half, and the BIR verifier requires all SBUF
        # operands of TensorScalarPtr to share a base partition.
        bqk_sw = const.tile([P, 4], F32)
        nc.sync.dma_start(bqk_sw, t["bqk4s"])

# Weight loads split per d-chunk so the first projection chain can
        # start as soon as its chunk lands (instead of waiting ~27us for
        # monolithic 4MB+ transfers). Ordered by first use.
        wqk_s = wpool.tile([P, ND, E4], F32R)
        wqk_r = t["wqkT"].rearrange("(k p) e -> p k e", p=P)
        for k in range(ND):
            nc.gpsimd.dma_start(wqk_s[:, k, :], wqk_r[:, k, :])
        wv_s = wpool.tile([P, ND, E2], F32R)
        wv_r = t["wvT"].rearrange("(k p) e -> p k e", p=P)
        for k in range(ND):
            nc.gpsimd.dma_start(wv_s[:, k, :], wv_r[:, k, :])
        wo_s = wpool.tile([P, HPC, D], BF16)
        nc.gpsimd.dma_start(wo_s, t["woT"].rearrange("(h p) d -> p h d", p=P))
        # rope tables go on the sync queue AFTER the first xt chunk (see
        # emit_A), so the first projection's input isn't queued behind 2MB of
        # tables; first rope needs them only ~2 proj units later.
        cosT = tabs.tile([P, S], F32)
        sinT = tabs.tile([P, S], F32)
        startup = [lambda: nc.scalar.dma_start(cosT, t["cosT"]),
                   lambda: nc.scalar.dma_start(sinT, t["sinT"])]

        def b_units(b, QT, KT, V, is_last=False):
            """Phase B (attention) + phase C (output proj) for one batch.

            Within a query chunk, exp(logits) production runs one key-tile
            ahead of its consumers (av matmul + Asum accumulate), so neither
            PE nor DVE ever waits on the ACT exp latency. The softmax
            denominator comes from an exact f32 Asum of the bf16 at tiles on
            DVE (bf16+bf16->f32 adds), finished by a single ones-matmul per
            (qc, h) on PE -- this removes the per-key-tile l matmuls that
            cost ~55us of PE time in the baseline.
            """
            deferred_y = []
            for qc in range(NQC):
                qsl = slice(qc * 512, (qc + 1) * 512)
                dqc = 2 * qc + 1
                av_ps, Asum, l_ps = {}, {}, {}
                ats_by_kt = {}

                def lg_exp(kt):
                    ats = {}
                    for h in range(HPC):
                        lg = psA.tile([P, 512], F32, tag="ps", name=f"lg_{b}_{qc}_{kt}_{h}")
                        nc.tensor.matmul(
                            lg, KT[:, h, kt * P:(kt + 1) * P], QT[:, h, qsl],
                            start=True, stop=True,
                        )
                        at = attnp.tile([P, 512], BF16, tag="at", name=f"at_{b}_{qc}_{kt}_{h}")
                        nc.scalar.activation(
                            at, lg, mybir.ActivationFunctionType.Exp, scale=SCALE
                        )
                        ats[h] = at
                    ats_by_kt[kt] = ats

                def consume(kt):
                    for h in range(HPC):
                        nc.tensor.matmul(
                            av_ps[h], V[:, kt, h * HD:(h + 1) * HD], ats_by_kt[kt][h],
                            start=(kt == 0), stop=(kt == NKT - 1),
                        )
                    for h in range(HPC):
                        # h0 chain on DVE, h1 on gpsimd (K_ASENG=split), or
                        # both on DVE: gpsimd's real f32 throughput is the
                        # least-trusted part of the cost model.
                        mode = os.environ.get("K_ASENG", "dve")
                        eng = nc.vector if (mode == "dve" or h == 0) else nc.gpsimd
                        if kt == 1:
                            eng.tensor_add(Asum[h], ats_by_kt[0][h], ats_by_kt[1][h])
                        elif kt > 1:
                            eng.tensor_add(Asum[h], Asum[h], ats_by_kt[kt][h])
                    if kt >= 2:
                        ats_by_kt.pop(kt - 2, None)

                LAG = int(os.environ.get("K_LAG", "2"))
                for kt in range(NKT):
                    # dep: QT/KT columns for this unit come from m-chunks
                    # <= max(2qc+1, kt//2) of this batch.
                    yield max(dqc, kt // 2)
                    if kt == 0:
                        for h in range(HPC):
                            av_ps[h] = psAv.tile([P, 512], F32, tag="av", name=f"av_{b}_{qc}_{h}")
                            Asum[h] = asump.tile([P, 512], F32R, tag="as", name=f"as_{b}_{qc}_{h}")
                    lg_exp(kt)
                    if kt >= LAG:
                        consume(kt - LAG)
                    # previous qc's deferred y tiles, one per unit, so their
                    # ACT evictions interleave with this qc's exp stream
                    # instead of bunching up at the final.
                    if deferred_y:
                        deferred_y.pop(0)()
                for kk in range(NKT - LAG, NKT):
                    yield max(dqc, (NKT - 1) // 2)
                    consume(kk)
                yield max(dqc, (NKT - 1) // 2)
                outc = outp.tile([P, HPC, 512], BF16, tag="outc", name=f"outc_{b}_{qc}")
                # av psum -> SBUF bf16 on ACT first: it only depends on the
                # last av matmul, so it overlaps the l/recip/rb chain, frees
                # the av bank early, and leaves rb as the only PSUM input of
                # the outc multiply.
                av_sb = {}
                for h in range(HPC):
                    av_sb[h] = avsbp.tile([P, 512], BF16, tag="avsb", name=f"avsb_{b}_{qc}_{h}")
                    nc.scalar.copy(av_sb[h], av_ps[h])
                # parallelize the two h-chains across engines: both l matmuls
                # back-to-back on PE, then both recips (DVE), rb broadcasts
                # (PE rank-1 matmul), outc muls (DVE).
                for h in range(HPC):
                    l_ps[h] = psA.tile([1, 512], F32, tag="ps", name=f"l_{b}_{qc}_{h}")
                    nc.tensor.matmul(l_ps[h], ones_c, Asum[h], start=True, stop=True)
                rss = {}
                use_pe_rb = os.environ.get("K_RB", "gp") == "pe"
                for h in range(HPC):
                    if use_pe_rb:
                        rss[h] = rsp.tile([1, 512], F32R, tag="rsr", name=f"rs_{b}_{qc}_{h}")
                        with nc.allow_low_precision(reason="f32r rounding of 1/l"):
                            nc.vector.reciprocal(rss[h], l_ps[h])
                    else:
                        rss[h] = rsp.tile([1, 512], F32, tag="rs", name=f"rs_{b}_{qc}_{h}")
                        nc.vector.reciprocal(rss[h], l_ps[h])
                rbs = {}
                for h in range(HPC):
                    if use_pe_rb:
                        # broadcast 1/l across partitions via a rank-1 PE
                        # matmul (gpsimd partition_broadcast is much slower on
                        # real hw than the cost model suggests, and it sits on
                        # the critical path to the y matmuls).
                        rbs[h] = psA.tile([P, 512], F32, tag="ps", name=f"rb_{b}_{qc}_{h}")
                        nc.tensor.matmul(rbs[h], ones_r, rss[h], start=True, stop=True)
                    else:
                        rbs[h] = rbp.tile([P, 512], F32, tag="rb", name=f"rb_{b}_{qc}_{h}")
                        nc.gpsimd.partition_broadcast(rbs[h], rss[h])
                for h in range(HPC):
                    nc.vector.tensor_mul(outc[:, h, :], av_sb[h], rbs[h])
                # pipeline distance: let a few projection units run on PE while
                # the recip->broadcast->outc chain completes, so the y matmuls
                # below don't stall the PE queue.
                for _ in range(int(os.environ.get("K_FYLD", "2"))):
                    yield max(dqc, (NKT - 1) // 2)

                def make_y(qc, outc, yu):
                    def emit_y():
                        mtl, och = divmod(yu, 2)
                        mt = qc * 4 + mtl
                        lsl = slice(mtl * P, (mtl + 1) * P)
                        for oc in (2 * och, 2 * och + 1):
                            osl = slice(oc * 512, (oc + 1) * 512)
                            y_ps = psA.tile([P, 512], F32, tag="ps", name=f"yp_{b}_{mt}_{oc}")
                            nc.tensor.matmul(
                                y_ps, outc[:, 0, lsl], wo_s[:, 0, osl],
                                start=True, stop=False,
                            )
                            nc.tensor.matmul(
                                y_ps, outc[:, 1, lsl], wo_s[:, 1, osl],
                                start=False, stop=True,
                            )
                            yt = yp.tile([P, 512], BF16, tag="yt", name=f"yt_{b}_{mt}_{oc}")
                            # in the final drain there is no projection work to
                            # hide behind, so split evictions ACT/DVE there.
                            if is_last and oc % 2 == 1:
                                nc.vector.tensor_copy(yt, y_ps)
                            else:
                                nc.scalar.copy(yt, y_ps)
                            nc.sync.dma_start(
                                t["y"][b * S + mt * P: b * S + (mt + 1) * P, osl], yt
                            )
                    return emit_y

                deferred_y = [make_y(qc, outc, yu) for yu in range(8)]
            # tail: the last qc's y tiles drain at the end of this batch's
            # generator, overlapping the next batch's projection stream.
            for _ in range(int(os.environ.get("K_TYLD", "3"))):
                yield NMC - 1
            for fn_ in deferred_y:
                yield NMC - 1
                fn_()

        def emit_A(b, pending, is_last=False):
            """Projections + rope for batch b. Each interleave slot first
            pumps the previous batch's attention stream (dependencies all
            met); once that is exhausted, it pulls forward THIS batch's own
            attention units whose Q/K/V chunks are already emitted, so the
            exp/attention stream starts before the projections finish."""
            QT = qtkv.tile([P, HPC, S], BF16, tag="QT", name=f"QT_{b}")
            KT = qtkv.tile([P, HPC, S], BF16, tag="KT", name=f"KT_{b}")
            V = qtkv.tile([P, NMT, E2], BF16, tag="V", name=f"V_{b}")
            own = _BStream(b_units(b, QT, KT, V, is_last=is_last))

            def slot(avail):
                if pending is not None and pending.pump(10**9):
                    return
                own.pump(avail)

            for mc in range(NMC):
                gm = b * S + mc * MC
                xt = xtp.tile([P, ND, MC], F32R, tag="xt", name=f"xt_{b}_{mc}")
                nc.sync.dma_start(
                    xt, t["xT"][:, gm:gm + MC].rearrange("(k p) m -> p k m", p=P).bitcast(F32R)
                )
                while startup:
                    startup.pop(0)()
                msl = slice(mc * MC, (mc + 1) * MC)
                for ecn in range(4):
                    ps = psA.tile([P, MC], F32, tag="ps", name=f"qk_{b}_{mc}_{ecn}")
                    for k in range(ND):
                        nc.tensor.matmul(
                            ps, wqk_s[:, k, ecn * P:(ecn + 1) * P], xt[:, k, :],
                            start=(k == 0), stop=(k == ND - 1),
                        )
                    slot(mc - 1)
                    dst = (QT if ecn < 2 else KT)[:, ecn % 2, msl]
                    bias = bqk_sb[:, ecn:ecn + 1]
                    # dst = (ps + bias) * cos ; u = swap_halves(ps + bias) * sin'
                    nc.vector.scalar_tensor_tensor(
                        dst, ps, bias, cosT[:, msl], ADD, MULT
                    )
                    u = up.tile([P, MC], BF16, tag="u", name=f"u_{b}_{mc}_{ecn}")
                    nc.vector.scalar_tensor_tensor(
                        u[0:H2, :], ps[H2:P, :], bqk_sw[0:H2, ecn:ecn + 1],
                        sinT[0:H2, msl], ADD, MULT,
                    )
                    nc.vector.scalar_tensor_tensor(
                        u[H2:P, :], ps[0:H2, :], bqk_sw[H2:P, ecn:ecn + 1],
                        sinT[H2:P, msl], ADD, MULT,
                    )
                    nc.vector.tensor_add(dst, dst, u)
                    slot(mc - 1)
                for mtl in range(MC // P):
                    mt = mc * (MC // P) + mtl
                    vps = psA.tile([P, E2], F32, tag="ps", name=f"v_{b}_{mt}")
                    for k in range(ND):
                        nc.tensor.matmul(
                            vps, xt[:, k, mtl * P:(mtl + 1) * P], wv_s[:, k, :],
                            start=(k == 0), stop=(k == ND - 1),
                        )
                    nc.scalar.copy(V[:, mt, :], vps)
                    slot(mc - 1)
            return own

        reps = int(os.environ.get("K_REPS", "1"))
        pending = None
        for rep in range(reps):
            for b in range(B):
                own = emit_A(b, pending,
                             is_last=(rep == reps - 1 and b == B - 1))
                if pending is not None:
                    while pending.pump(10**9):  # drain any remainder
                        pass
                pending = own
        while pending.pump(10**9):
            pass


def build_program():
    nc = bacc.Bacc(
        "TRN2",
        target_bir_lowering=False,
        debug=False,
        enable_asserts=False,
        num_devices=N_CORES,
    )
    t = {
        "xT": nc.dram_tensor("xT", [D, M], F32, kind="ExternalInput").ap(),
        "wqkT": nc.dram_tensor("wqkT", [D, E4], F32, kind="ExternalInput").ap(),
        "wvT": nc.dram_tensor("wvT", [D, E2], F32, kind="ExternalInput").ap(),
        "woT": nc.dram_tensor("woT", [E2, D], BF16, kind="ExternalInput").ap(),
        "bqk4": nc.dram_tensor("bqk4", [P, 4], F32, kind="ExternalInput").ap(),
        "bqk4s": nc.dram_tensor("bqk4s", [P, 4], F32, kind="ExternalInput").ap(),
        "cosT": nc.dram_tensor("cosT", [P, S], F32, kind="ExternalInput").ap(),
        "sinT": nc.dram_tensor("sinT", [P, S], F32, kind="ExternalInput").ap(),
        "y": nc.dram_tensor("y", [M, D], BF16, kind="ExternalOutput").ap(),
    }
    with tile.TileContext(nc) as tc:
        _emit(nc, tc, t)
    nc.compile()
    return nc


def rope_tables_T():
    inv_freq = 1.0 / (ROPE_THETA ** (np.arange(0, HD, 2, dtype=np.float32) / HD))
    angles = np.outer(np.arange(S, dtype=np.float32), inv_freq)  # [S, 64]
    ang = np.concatenate([angles, angles], axis=-1)  # [S, 128]
    cos = np.cos(ang).astype(np.float32)
    sin = np.sin(ang).astype(np.float32)
    sins = np.concatenate([-sin[:, :H2], sin[:, H2:]], axis=-1)
    return np.ascontiguousarray(cos.T), np.ascontiguousarray(sins.T)


def make_in_maps(x, wq, bq, wk, bk, wv, bv, wo, bo):
    import ml_dtypes
    xT = np.ascontiguousarray(x.reshape(M, D).T)
    cosT, sinT = rope_tables_T()
    maps = []
    for c in range(N_CORES):
        sl = slice(c * E2, (c + 1) * E2)
        bqk = np.concatenate([bq[sl], bk[sl]]).astype(np.float32)
        maps.append({
            "xT": xT,
            "wqkT": np.ascontiguousarray(np.concatenate([wq[sl], wk[sl]], axis=0).T),
            "wvT": np.ascontiguousarray(wv[sl].T),
            "woT": np.ascontiguousarray(wo[:, sl].T).astype(ml_dtypes.bfloat16),
            "bqk4": np.ascontiguousarray(bqk.reshape(4, P).T),
            "bqk4s": np.ascontiguousarray(np.roll(bqk.reshape(4, P), H2, axis=1).T),
            "cosT": cosT,
            "sinT": sinT,
        })
    return maps


_NC = None


def kernel(**inputs) -> np.ndarray:
    global _NC
    inputs = {k: np.ascontiguousarray(np.asarray(v, dtype=np.float32))
              for k, v in inputs.items()}
    if _NC is None:
        _NC = build_program()
    from concourse.bass_utils import run_bass_kernel_spmd

    maps = make_in_maps(**inputs)
    res = run_bass_kernel_spmd(_NC, maps, list(range(N_CORES)))
    y = np.zeros((M, D), np.float64)
    for c in range(N_CORES):
        y += np.asarray(res.results[c]["y"], dtype=np.float64)
    y += inputs["bo"][None, :] + (inputs["bv"].astype(np.float64) @ inputs["wo"].T.astype(np.float64))[None, :]
    return y.astype(np.float32).reshape(B, S, D)
